# revision 55
# baseline (speedup 1.0000x reference)
"""MixtureAttention (MoE attention routing) Trainium2 kernel — routed, bf16.

Strategy: expert-parallel over 8 NeuronCores (one expert per core) with
HOST-SIDE top-2 routing.  The router (logits = q @ Wr + br, top-2 +
softmax) runs in numpy on the host; only the ~N*K/E tokens that actually
selected each expert are gathered (with capacity padding, CAP=576 per
(batch, expert)) and shipped to that expert's core.  This cuts device
work ~4x vs computing every expert for every token.  Padded slots carry
router weight 0 so they contribute nothing; the host scatters per-core
outputs back with a fancy-index add.

Routing safety: top-2 selection is stable (min 2nd-vs-3rd logit gap
~1.6e-4 for the reference data, far above f32 noise).  If any
(expert,batch) exceeds CAP, the host runs extra rounds (outputs add).

All matmul operands are bf16 (PE peak rate; f32 PSUM accumulation);
empirically rel err ~6e-3 vs the f64 reference (tolerance 2e-2).  Host
pre-converts inputs/weights to bf16 (halves DMA), pre-folds the
attention scale into wq/bq, and pre-shards qT/kT/vT/o so every DMA line
is >=5KB contiguous.  DMA triggers are spread over the Pool (wk/wv),
ACT (kT/vT, wo, out) and SP (wq, qTc, denom gathers) queues so
transfers parallelize — on HW a single queue serializes against compute.

Device kernel per core (expert e), T-layout (feature on partitions,
token on free dim), per batch b:
  KT = wk^T kT + bk;  V = (vT^T wv + bv) with a ones column per head
  (65th) so the AV matmul also produces the softmax denominator row.
  per 288-token chunk of the CAP gathered slots:
    Q = wq'^T qT + bq'      (hoisted into the PREVIOUS chunk's heads
                             phase so PE has work while ACT runs exp)
    head pairs hp (head 2hp on partitions 0-63, 2hp+1 on 64-127 —
    adjacent S matmuls hit disjoint PE row groups and run concurrently):
      S^T = K_h^T Q_h -> exp (ACT, both heads batched) -> AV+denom (PE)
      -> copy AV out + stage denom row (DVE)
    after each 8-head half: gather denom rows (DMA), one batched
    reciprocal (DVE — reciprocal is an 8 cyc/elem iterative divide, so
    per-head [1,NQC] recips are ~7x more expensive than one [8,NQC]),
    then per-ko selector matmuls replicate recip rows across partition
    halves (PE does the partition-broadcast) -> normalize mult (DVE).
    O-proj: wo^T O + bo (DVE) -> * router weight (DVE) -> out DMA (bf16).
  The b+1 K/V projection is hoisted into b's last heads phase (KT/V
  double-buffered).
"""

import numpy as np

B, N, D, E, H = 2, 2048, 1024, 8, 16
MK = 512            # keys/values chunk per expert (M // E)
HD = D // H         # 64
P = 128
KO = D // P         # 8
CAP = 576           # per (batch, expert) token-slot capacity
NQC = 288           # token chunk (matmul free dim)
NCH = CAP // NQC    # 2
SCALE = HD ** -0.5
TOPK = 2
CORES = 8

_NC = None
import os
ABL = set(os.environ.get("KABL", "").split(","))
KREP = int(os.environ.get("KREP", "1"))


def _build_nc():
    import concourse.bacc as bacc
    import concourse.mybir as mybir
    from concourse.tile import TileContext

    f32 = mybir.dt.float32
    f32r = mybir.dt.float32r
    bf16 = mybir.dt.bfloat16
    Af = mybir.ActivationFunctionType
    Op = mybir.AluOpType

    nc = bacc.Bacc("TRN2", target_bir_lowering=False)

    qT_d = nc.declare_dram_parameter("qT", [B, NCH, P, KO, NQC], bf16,
                                    isOutput=False)
    kT_d = nc.declare_dram_parameter("kT", [B, P, KO, MK], bf16,
                                    isOutput=False)
    vT_d = nc.declare_dram_parameter("vT", [B, P, KO, MK], bf16,
                                    isOutput=False)
    wq_d = nc.declare_dram_parameter("wq", [D, D], bf16, isOutput=False)
    wk_d = nc.declare_dram_parameter("wk", [D, D], bf16, isOutput=False)
    wv_d = nc.declare_dram_parameter("wv", [D, D], bf16, isOutput=False)
    wo_d = nc.declare_dram_parameter("wo", [D, D], bf16, isOutput=False)
    bq_d = nc.declare_dram_parameter("bq", [D], f32, isOutput=False)
    bk_d = nc.declare_dram_parameter("bk", [D], f32, isOutput=False)
    bv_d = nc.declare_dram_parameter("bv", [D], bf16, isOutput=False)
    bo_d = nc.declare_dram_parameter("bo", [D], f32, isOutput=False)
    w_d = nc.declare_dram_parameter("w", [B, CAP], bf16, isOutput=False)
    sels_d = nc.declare_dram_parameter("sels", [H // 2, KO // 2, P], f32r,
                                       isOutput=False)
    o_d = nc.declare_dram_parameter("o", [B, NCH, P, KO, NQC], bf16,
                                   isOutput=True)

    wq_r = wq_d.rearrange("(ki p) o -> p ki o", p=P)
    wk_r = wk_d.rearrange("(ki p) o -> p ki o", p=P)
    wv_r = wv_d.rearrange("(ki p) o -> p ki o", p=P)
    wo_r = wo_d.rearrange("(ki p) o -> p ki o", p=P)

    import concourse.bass as bass

    def pbcast(ap, nparts):
        # partition-stride-0 DMA source: replicate a [..] dram vector to
        # nparts partitions
        return bass.AP(tensor=ap.tensor, offset=ap.offset,
                       ap=[[0, nparts]] + list(ap.ap))

    with TileContext(nc) as tc:
        with tc.tile_pool(name="const", bufs=1) as cst, \
             tc.tile_pool(name="kvlong", bufs=1) as kvl, \
             tc.tile_pool(name="kvstage", bufs=2) as kvs, \
             tc.tile_pool(name="psp", bufs=1, space="PSUM") as psp:

            ones32 = cst.tile([P, P], f32, tag="ones32")
            nc.vector.memset(ones32[:], 1.0)
            # head-pair selectors (host constant): SELS[:, ko]^T @ recs
            # replicates recip row 2ko across partitions 0-63 and row 2ko+1
            # across 64-127 in one matmul (PE does the partition-broadcast)
            sels = cst.tile([H // 2, KO // 2, P], f32r, tag="sels")
            nc.sync.dma_start(sels[:], sels_d[:])

            wq_sb = cst.tile([P, KO, D], bf16, tag="wq")
            wk_sb = cst.tile([P, KO, D], bf16, tag="wk")
            wv_sb = cst.tile([P, KO, D], bf16, tag="wv")
            wo_sb = cst.tile([P, KO, D], bf16, tag="wo")
            bq_sb = cst.tile([P, KO], f32, tag="bq")
            bk_sb = cst.tile([P, KO], f32, tag="bk")
            bo_sb = cst.tile([P, KO], f32, tag="bo")
            bv_bc = cst.tile([P, D], bf16, tag="bv")
            w_row = cst.tile([1, B, CAP], bf16, tag="w_row")
            w_bc = cst.tile([P, B, CAP], bf16, tag="w_bc")

            KTs = [kvl.tile([P, KO, MK], bf16, tag="KT", name=f"KT{i}",
                            bufs=2) for i in range(B)]
            Vs = [kvl.tile([P, MK // P, H * (HD + 1)], bf16, tag="V",
                           name=f"V{i}", bufs=2) for i in range(B)]

            import contextlib
            rep_ctx = (tc.For_i(0, KREP, 1) if KREP > 1
                       else contextlib.nullcontext())
            with rep_ctx:
              # DMAs distributed over engine queues so transfers parallelize:
              # weights on the Pool queue, kT/vT on the ACT queue, qTc/out on
              # the SP queue.  K-path first so K-proj can start ASAP.
              nc.gpsimd.dma_start(wk_sb[:], wk_r[:])
              nc.sync.dma_start(bk_sb[:], bk_d.rearrange("(ko p) -> p ko", p=P))
              kTs = [kvs.tile([P, KO, MK], bf16, tag="kTs", name=f"kTs{i}")
                     for i in range(B)]
              vTs = [kvs.tile([P, KO, MK], bf16, tag="vTs", name=f"vTs{i}")
                     for i in range(B)]
              nc.scalar.dma_start(kTs[0][:, :KO // 2], kT_d[0, :, :KO // 2])
              nc.scalar.dma_start(kTs[0][:, KO // 2:], kT_d[0, :, KO // 2:])
              nc.gpsimd.dma_start(wv_sb[:], wv_r[:])
              nc.gpsimd.dma_start(bv_bc[:], pbcast(bv_d[:], P))
              nc.scalar.dma_start(vTs[0][:], vT_d[0])
              nc.sync.dma_start(wq_sb[:], wq_r[:])
              nc.sync.dma_start(bq_sb[:], bq_d.rearrange("(ko p) -> p ko", p=P))
              nc.sync.dma_start(w_row[:], pbcast(w_d[:], 1))
              nc.gpsimd.partition_broadcast(
                  w_bc[:].rearrange("p b t -> p (b t)"),
                  w_row[:].rearrange("o b t -> o (b t)"))
              nc.scalar.dma_start(kTs[1][:], kT_d[1])
              nc.scalar.dma_start(vTs[1][:], vT_d[1])
              nc.scalar.dma_start(wo_sb[:], wo_r[:])
              nc.sync.dma_start(bo_sb[:], bo_d.rearrange("(ko p) -> p ko", p=P))
              with tc.tile_pool(name="chunk", bufs=2) as chk, \
                   tc.tile_pool(name="pt_pool", bufs=4) as ptp, \
                   tc.tile_pool(name="fin_pool", bufs=2) as fpl:

                  chunks = [(b, c) for b in range(B) for c in range(NCH)]

                  def emit_qproj(i):
                      # Q projection for chunk i (scale pre-folded on host).
                      # Hoisted into the previous chunk's heads phase so the
                      # PE has work while ACT runs exp.
                      b, c = chunks[i]
                      tok0 = c * NQC
                      qTc = chk.tile([P, KO, NQC], bf16, tag="qTc", bufs=2,
                                     name=f"qTc_{i}")
                      nc.sync.dma_start(qTc[:], qT_d[b, c])
                      Qc = chk.tile([P, KO, NQC], bf16, tag="Qc", bufs=2,
                                    name=f"Qc_{i}")
                      for ko in range(KO):
                          pq = psp.tile([P, 512], f32, tag="big", bufs=2,
                                        name=f"pq_{i}_{ko}")
                          for ki in range(KO):
                              nc.tensor.matmul(
                                  pq[:, :NQC],
                                  wq_sb[:, ki, ko * P:(ko + 1) * P],
                                  qTc[:, ki],
                                  start=(ki == 0), stop=(ki == KO - 1))
                          nc.vector.tensor_scalar(
                              Qc[:, ko], pq[:, :NQC], bq_sb[:, ko:ko + 1],
                              None, Op.add)
                      return Qc

                  def emit_kv(b):
                      # ---- K/V projection for batch b ----
                      kT, vT = kTs[b], vTs[b]
                      KT, V = KTs[b], Vs[b]
                      # KT = wk^T @ kT + bk  (dout on partitions, mk free)
                      for ko in range(KO):
                          pk = psp.tile([P, MK], f32, tag="big", bufs=2)
                          for ki in range(KO):
                              nc.tensor.matmul(
                                  pk[:], wk_sb[:, ki, ko * P:(ko + 1) * P],
                                  kT[:, ki],
                                  start=(ki == 0), stop=(ki == KO - 1))
                          nc.vector.tensor_scalar(
                              KT[:, ko], pk[:], bk_sb[:, ko:ko + 1], None,
                              Op.add)
                      # V natural [mk, dout] = vT^T @ wv + bv, interleaved
                      # with a ones column every HD+1 so AV also produces the
                      # softmax sum
                      vview = V[:].rearrange("p m (h c) -> p m h c", c=HD + 1)
                      nc.vector.tensor_copy(
                          vview[:, :, :, HD],
                          ones32[:, :(MK // P) * H].rearrange(
                              "p (m h) -> p m h", m=MK // P))
                      for half in range(2):
                          for mt in range(MK // P):
                              pv = psp.tile([P, D // 2], f32, tag="big",
                                            bufs=2)
                              for ki in range(KO):
                                  nc.tensor.matmul(
                                      pv[:], vT[:, ki, mt * P:(mt + 1) * P],
                                      wv_sb[:, ki, half * (D // 2):
                                            (half + 1) * (D // 2)],
                                      start=(ki == 0), stop=(ki == KO - 1))
                              hsl = slice(half * (H // 2),
                                          (half + 1) * (H // 2))
                              nc.vector.tensor_tensor(
                                  vview[:, mt, hsl, :HD],
                                  pv[:].rearrange("p (h c) -> p h c", c=HD),
                                  bv_bc[:, half * (D // 2):
                                        (half + 1) * (D // 2)]
                                  .rearrange("p (h c) -> p h c", c=HD),
                                  Op.add)

                  qcs = {}
                  for b in range(B):
                      if b == 0:
                          emit_kv(0)
                      KT, V = KTs[b], Vs[b]

                      # ---- chunk loop ----
                      for c in range(NCH):
                          i = b * NCH + c
                          tok0 = c * NQC
                          if i == 0:
                              qcs[0] = emit_qproj(0)
                          Qc = qcs[i]

                          # ---- heads (paired: head 2hp on partitions 0-63,
                          # head 2hp+1 on 64-127; adjacent S matmuls target
                          # disjoint PE row groups and run concurrently) ----
                          O_sb = chk.tile([P, KO, NQC], bf16, tag="O_sb", bufs=2)
                          densq = ptp.tile([P, H // 2, NQC], f32, tag="densq",
                                           bufs=1)

                          for hp in range(H // 2):
                              pos = [psp.tile([HD + 1, 512], f32, tag="po",
                                              bufs=2, name=f"po_{hp}_{d}")
                                     for d in range(2)]
                              for mt in range(MK // P):
                                  # both heads' S tiles side by side; the two
                                  # matmuls hit disjoint row groups -> overlap
                                  ps2 = psp.tile([P, 2, 512], f32, tag="ps2",
                                                 bufs=2,
                                                 name=f"ps2_{hp}_{mt}")
                                  for d in range(2):
                                      p0 = d * HD
                                      nc.tensor.matmul(
                                          ps2[:, d, :NQC],
                                          KT[p0:p0 + HD, hp,
                                             mt * P:(mt + 1) * P],
                                          Qc[p0:p0 + HD, hp],
                                          start=True, stop=True)
                                  pe2 = ptp.tile([P, 2, NQC], bf16,
                                                 tag="pe", bufs=4,
                                                 name=f"pe_{hp}_{mt}")
                                  nc.scalar.activation(
                                      pe2[:], ps2[:, :, :NQC],
                                      Af.Copy if "noexp" in ABL else Af.Exp)
                                  for d in range(2):
                                      h = 2 * hp + d
                                      nc.tensor.matmul(
                                          pos[d][:, :NQC],
                                          V[:, mt,
                                            h * (HD + 1):(h + 1) * (HD + 1)],
                                          pe2[:, d],
                                          start=(mt == 0),
                                          stop=(mt == MK // P - 1))
                              if hp == 0 and i + 1 < len(chunks):
                                  qcs[i + 1] = emit_qproj(i + 1)
                              if hp == 1 and c == NCH - 1 and b + 1 < B:
                                  emit_kv(b + 1)
                              for d in range(2):
                                  po = pos[d]
                                  p0 = d * HD
                                  # unnormalized head output + its denom row;
                                  # po releases after these two copies
                                  nc.vector.tensor_copy(
                                      O_sb[p0:p0 + HD, hp], po[:HD, :NQC])
                                  if "nonorm" not in ABL:
                                      # stage denom rows on partition 0 at
                                      # per-head free offsets (engines cannot
                                      # write non-32-aligned partitions)
                                      h = 2 * hp + d
                                      base = HD * (h // 8)
                                      nc.vector.tensor_copy(
                                          densq[base:base + 1, h % 8, :],
                                          po[HD:HD + 1, :NQC])

                              if "nonorm" not in ABL and hp % 4 == 3:
                                  # normalize the finished half: the gather
                                  # DMA packs 8 denom rows as [64, 40] so the
                                  # reciprocal (8 cyc/elem iterative divide)
                                  # runs on 64 lanes; a second DMA lays the
                                  # recips back out as [8, NQC] rows for the
                                  # selector matmuls (PE partition-broadcast)
                                  g = hp // 4
                                  densg = ptp.tile([H // 2, NQC], f32,
                                                   tag="densg", bufs=2,
                                                   name=f"densg_{hp}")
                                  nc.gpsimd.dma_start(
                                      densg[:],
                                      densq[g * HD:g * HD + 1, :, :])
                                  recs = ptp.tile([H // 2, NQC], f32r,
                                                  tag="recs", bufs=2,
                                                  name=f"recs_{hp}")
                                  with nc.allow_low_precision(
                                          reason="softmax denom recip"):
                                      nc.vector.reciprocal(recs[:], densg[:])
                                  for k4 in range(KO // 2):
                                      ko = 4 * g + k4
                                      rbp = psp.tile([P, 512], f32, tag="big",
                                                     bufs=2,
                                                     name=f"rbp_{ko}")
                                      nc.tensor.matmul(
                                          rbp[:, :NQC], sels[:, k4],
                                          recs[:], start=True, stop=True)
                                      nc.vector.tensor_tensor(
                                          O_sb[:, ko], O_sb[:, ko],
                                          rbp[:, :NQC], Op.mult)

                          # ---- output projection + bias + router weight ----
                          fin = fpl.tile([P, KO, NQC], bf16, tag="fin",
                                         name=f"fin_{i}")
                          for ko in range(KO):
                              pf = psp.tile([P, 512], f32, tag="big", bufs=2)
                              for ki in range(KO):
                                  nc.tensor.matmul(
                                      pf[:, :NQC], wo_sb[:, ki, ko * P:(ko + 1) * P],
                                      O_sb[:, ki],
                                      start=(ki == 0), stop=(ki == KO - 1))
                              nc.vector.tensor_scalar(
                                  fin[:, ko], pf[:, :NQC], bo_sb[:, ko:ko + 1],
                                  None, Op.add)
                              nc.vector.tensor_tensor(
                                  fin[:, ko], fin[:, ko],
                                  w_bc[:, b, tok0:tok0 + NQC], Op.mult)
                          nc.scalar.dma_start(o_d[b, c], fin[:])
    nc.finalize()
    return nc


def _get_nc():
    global _NC
    if _NC is None:
        _NC = _build_nc()
    return _NC


def route(inputs):
    """Host-side top-2 routing.

    Returns (idx, wts, counts, nrounds): idx/wts are [E, B, nrounds*CAP]
    token indices and router weights (padded with idx 0 / weight 0).
    """
    q = np.asarray(inputs["queries"], dtype=np.float32)
    Wr = np.asarray(inputs["Wr"], dtype=np.float32)
    br = np.asarray(inputs["br"], dtype=np.float32)
    logits = q @ Wr + br                                    # [B, N, E]
    top2 = np.argsort(-logits, axis=-1, kind="stable")[..., :TOPK]
    tv = np.take_along_axis(logits, top2, axis=-1)
    ex = np.exp(tv - tv.max(-1, keepdims=True))
    rw = ex / ex.sum(-1, keepdims=True)                     # [B, N, K]

    counts = np.zeros((E, B), dtype=np.int64)
    for b in range(B):
        for k in range(TOPK):
            np.add.at(counts[:, b], top2[b, :, k], 1)
    nrounds = int(max(1, -(-counts.max() // CAP)))
    S = nrounds * CAP
    idx = np.zeros((E, B, S), dtype=np.int64)
    wts = np.zeros((E, B, S), dtype=np.float32)
    for b in range(B):
        for e in range(E):
            sel = np.nonzero((top2[b] == e).any(axis=-1))[0]
            we = np.where(top2[b, sel, 0] == e, rw[b, sel, 0], rw[b, sel, 1])
            idx[e, b, :len(sel)] = sel
            wts[e, b, :len(sel)] = we
    return idx, wts, counts, nrounds


def build_in_maps(inputs, idx, wts, r0):
    """Per-core inputs for routing round r0 (slots r0*CAP..(r0+1)*CAP)."""
    from ml_dtypes import bfloat16

    ins = {k: np.asarray(v, dtype=np.float32) for k, v in inputs.items()}
    sl = slice(r0 * CAP, (r0 + 1) * CAP)
    kTb = ins["keys"].transpose(0, 2, 1).astype(bfloat16)    # [B, D, M]
    vTb = ins["values"].transpose(0, 2, 1).astype(bfloat16)

    def shard_kv(x):
        # [D, MK] -> [P, KO, MK] with d = ki*P + p
        return np.ascontiguousarray(
            x.reshape(KO, P, MK).transpose(1, 0, 2))
    sels = np.zeros((H // 2, KO // 2, P), dtype=np.float32)
    for k4 in range(KO // 2):
        sels[2 * k4, k4, 0:HD] = 1.0
        sels[2 * k4 + 1, k4, HD:P] = 1.0
    in_maps = []
    for e in range(CORES):
        qg = np.empty((B, NCH, P, KO, NQC), dtype=bfloat16)
        for b in range(B):
            # [D, CAP] -> [NCH, P, KO, NQC] with d = ki*P + p, t = c*NQC + tq
            qg[b] = (ins["queries"][b, idx[e, b, sl]].T
                     .reshape(KO, P, NCH, NQC).transpose(2, 1, 0, 3))
        in_maps.append({
            "qT": qg,
            "kT": np.stack([shard_kv(kTb[b, :, e * MK:(e + 1) * MK])
                            for b in range(B)]),
            "vT": np.stack([shard_kv(vTb[b, :, e * MK:(e + 1) * MK])
                            for b in range(B)]),
            "wq": (ins["Wq"][e] * SCALE).astype(bfloat16),
            "wk": ins["Wk"][e].astype(bfloat16),
            "wv": ins["Wv"][e].astype(bfloat16),
            "wo": ins["Wo"][e].astype(bfloat16),
            "bq": ins["bq"][e] * SCALE, "bk": ins["bk"][e],
            "bv": ins["bv"][e].astype(bfloat16), "bo": ins["bo"][e],
            "w": np.ascontiguousarray(wts[e, :, sl]).astype(bfloat16),
            "sels": sels,
        })
    return in_maps


def combine(results, idx, counts, r0, out):
    """Scatter-add one round's per-core outputs into out [B, N, D].

    Valid slots per (e, b) this round: counts[e, b] clipped to the round's
    CAP window; tokens within an expert are unique so fancy-index add works.
    """
    for e in range(CORES):
        oe = np.asarray(results[e]["o"], dtype=np.float32)
        # [B, NCH, P, KO, NQC] -> [B, CAP, D] with d = ki*P + p
        oe = oe.transpose(0, 1, 4, 3, 2).reshape(B, CAP, D)
        for b in range(B):
            n = int(min(max(counts[e, b] - r0 * CAP, 0), CAP))
            if n == 0:
                continue
            tok = idx[e, b, r0 * CAP:r0 * CAP + n]
            out[b, tok] += oe[b, :n]
    return out


def kernel(**inputs) -> np.ndarray:
    from concourse.bass_utils import run_bass_kernel_spmd

    idx, wts, counts, nrounds = route(inputs)
    nc = _get_nc()
    out = np.zeros((B, N, D), dtype=np.float32)
    for r0 in range(nrounds):
        in_maps = build_in_maps(inputs, idx, wts, r0)
        res = run_bass_kernel_spmd(nc, in_maps, list(range(CORES))).results
        combine(res, idx, counts, r0, out)
    return out
